# revision 1
# baseline (speedup 1.0000x reference)
"""Trainium2 Bass kernel for BSplineBasis (degree-3, 64 uniform-ish knots).

Math: the reference evaluates, for each normalized point xn and each of 60
basis elements i, a piecewise cubic (de Boor with clamped interval index).
With simple inner knots this is exactly representable in truncated-power form:

    out[n, i] = sum_q A[q,i] * xn^q  +  sum_m J[m,i] * relu(xn - kappa_m)^3

with 56 inner-knot features kappa_m and a banded (3 taps/column) jump table J.
On device (per 490-point tile, two tiles paired via block-diagonal weights so
fp32r matmuls keep start_partition 0 and DVE/ACT passes use 120 lanes):
  MM1 (K=6, fp32r): unclamped cube polynomials + y-power pass-throughs from
      rows [y, y^2, y^3] (y = xn - 0.5, centered to tame monomial
      cancellation at tf32 precision); constant terms live in the clamp bias
  clamp (DVE): stack = max(G + bias, minclamp) with per-partition minclamp
      (-inf on the power rows so negative y passes through)
  MM2 (K=120, fp32r): out.T = blockdiag(W2, W2).T @ stack -> PSUM
  evict (ACT): PSUM -> SBUF staging, then HWDGE DMA to DRAM out_t [60, shard]
Each core redundantly computes the global min/max from the full x (no
collective needed) and processes a 62,500-point shard; the host transposes
out_t back and patches the (rare) xn == 1.0 rows where the reference jumps
to its degenerate right-end pieces.

Tables are derived from the knots at runtime on the host in float64, by
fitting the reference's own de Boor piece recursion (exact for cubics).
"""
import os
import sys

import numpy as np

if "/opt/trn_rl_repo" not in sys.path:
    sys.path.insert(0, "/opt/trn_rl_repo")

DEGREE = 3
NUM_KNOTS = 64
NB = NUM_KNOTS - DEGREE - 1          # 60 basis elements
N_POINTS = 500_000
N_CORES = 8
SHARD = N_POINTS // N_CORES          # 62500
TILE_W = 490                          # points per matmul tile (even: fp32r)
N_TILES = 128                         # 128 * 490 = 62720 >= SHARD
SHARD_PAD = N_TILES * TILE_W          # 62720
NF = 56                               # truncated-power features
FULL_COLS = 3907                      # 128 * 3907 = 500096 >= N_POINTS
FULL_PAD = 128 * FULL_COLS


# ----------------------------------------------------------------- host math
def _piece_poly_coeffs(knots, i, ell):
    """Monomial coeffs (len 4) of the de Boor piece for element i, interval
    ell in [3,6] — replicates the reference recursion, fit exactly in f64."""
    k = DEGREE
    seg = knots[i:i + k + 2]
    T = np.concatenate([np.full(k, seg[0] - 1.0), seg, np.full(k, seg[-1] + 1.0)])

    def eval_at(x):
        res = [np.float64(1.0)] + [np.float64(0.0)] * k
        for j in range(1, k + 1):
            hh = list(res[:j])
            res[0] = np.float64(0.0)
            for n in range(1, j + 1):
                tb, ta = T[ell + n], T[ell + n - j]
                den = tb - ta
                w = 0.0 if den == 0 else hh[n - 1] / den
                res[n - 1] = res[n - 1] + w * (tb - x)
                res[n] = w * (x - ta)
        return res[2 * k - ell]

    xs = np.linspace(-0.3, 1.3, 5)
    V = np.vander(xs, 4, increasing=True)
    return np.linalg.lstsq(V, np.array([eval_at(x) for x in xs]), rcond=None)[0]


def build_tables(knots):
    """A [4,60], CUBE4 [4,56], J [56,60] for the truncated-power form."""
    knots = np.asarray(knots, np.float64)
    P = [[_piece_poly_coeffs(knots, i, p + 3) for p in range(4)] for i in range(NB)]

    def p_of(s, i):
        return int(np.clip(s - i - 1, 0, 3))

    A = np.zeros((4, NB))
    for i in range(NB):
        A[:, i] = P[i][p_of(4, i)]

    ms = list(range(4, 60))
    J = np.zeros((len(ms), NB))
    for f, m in enumerate(ms):
        for i in range(NB):
            pb, pa = p_of(m, i), p_of(m + 1, i)
            if pa != pb:
                J[f, i] = (P[i][pa] - P[i][pb])[3]

    kaps = knots[4:60]
    CUBE4 = np.stack([-kaps**3, 3 * kaps**2, -3 * kaps, np.ones(NF)], 0)
    # reference row at xn == 1.0 exactly: searchsorted gives s = 64 there,
    # so every column evaluates its piece p=3 at 1.0 (a genuine jump for the
    # right-boundary columns); patched on the host for the (rare) argmax hits
    row1 = np.array([np.polyval(P[i][3][::-1], 1.0) for i in range(NB)])
    return A, CUBE4, J, row1


CENTER = 0.5  # powers are of y = xn - CENTER to reduce monomial cancellation


def _shift_poly(c, h):
    """coeffs of p(y + h) given coeffs c of p(x), low->high, exact in f64."""
    from math import comb
    out = np.zeros_like(c)
    for q in range(4):
        for r in range(q + 1):
            out[r] += c[q] * comb(q, r) * h ** (q - r)
    return out


def _make_const_arrays(knots):
    A, CUBE4, J, row1 = build_tables(knots)
    # re-express in y = xn - CENTER
    A = np.stack([_shift_poly(A[:, i], CENTER) for i in range(NB)], 1)
    kaps = np.asarray(knots, np.float64)[4:60] - CENTER
    CUBE4 = np.stack([-kaps**3, 3 * kaps**2, -3 * kaps, np.ones(NF)], 0)
    # Two point-tiles are processed per matmul via block-diagonal weights
    # (fp32r requires output start_partition 0; block-diag gives M=120).
    # MM1 (K=6: powers of tile a, powers of tile b): cols 0-55 produce the
    # non-constant part of (xn-kappa)^3, cols 56-59 pass powers through
    # (constant terms come in via the clamp bias).
    c3 = np.zeros((3, NB), np.float32)
    c3[:, :NF] = CUBE4[1:4, :]
    for q in range(1, 4):
        c3[q - 1, NF + q] = 1.0
    cube3x = np.zeros((6, 2 * NB), np.float32)
    cube3x[0:3, :NB] = c3
    cube3x[3:6, NB:] = c3
    bias = np.zeros((2 * NB, 1), np.float32)
    clampc = np.zeros((2 * NB, 2), np.float32)  # col0: DVE min-clamp, col1: ACT alpha
    clampc[:, 0] = -3.0e38
    for h in (0, NB):
        bias[h:h + NF, 0] = CUBE4[0, :]  # -kappa'^3
        bias[h + NF, 0] = 1.0            # the y^0 == 1 row
        clampc[h:h + NF, 0] = 0.0        # cube rows: relu
        clampc[h + NF:h + NB, 1] = 1.0   # power rows: identity (alpha=1)
    # MM2 weights: rows 0-55 = J band, rows 56-59 = base cubic A, blockdiag
    w2s = np.zeros((NB, NB), np.float32)
    w2s[:NF, :] = J
    w2s[NF:, :] = A
    w2 = np.zeros((2 * NB, 2 * NB), np.float32)
    w2[:NB, :NB] = w2s
    w2[NB:, NB:] = w2s
    return cube3x, bias, clampc, w2, row1


# -------------------------------------------------------------- bass program
_CACHE = {}


def _build_nc():
    import concourse.tile as tile
    from concourse import bacc, mybir

    f32 = mybir.dt.float32
    f32r = mybir.dt.float32r

    nc = bacc.Bacc("TRN2", target_bir_lowering=False, debug=False)
    x_full = nc.declare_dram_parameter("x_full", [128, FULL_COLS], f32, isOutput=False)
    x_shard = nc.declare_dram_parameter("x_shard", [128, TILE_W], f32, isOutput=False)
    cube3x_d = nc.declare_dram_parameter("cube3x", [6, 2 * NB], f32, isOutput=False)
    bias_d = nc.declare_dram_parameter("bias124", [2 * NB, 1], f32, isOutput=False)
    clampc_d = nc.declare_dram_parameter("clampc", [2 * NB, 2], f32, isOutput=False)
    w2_d = nc.declare_dram_parameter("w2", [2 * NB, 2 * NB], f32, isOutput=False)
    out_t = nc.declare_dram_parameter("out_t", [NB, SHARD_PAD], f32, isOutput=True)

    GROUP = 8  # pairs per output staging buffer

    with tile.TileContext(nc) as tc:
        with (
            tc.tile_pool(name="big", bufs=1) as big_pool,
            tc.tile_pool(name="consts", bufs=1) as const_pool,
            tc.tile_pool(name="xrows", bufs=6) as xrows_pool,
            tc.tile_pool(name="stack", bufs=4) as stack_pool,
            tc.tile_pool(name="stage", bufs=3) as stage_pool,
            tc.tile_pool(name="gpsum", bufs=2, space="PSUM") as gpsum_pool,
            tc.tile_pool(name="opsum", bufs=2, space="PSUM") as opsum_pool,
            tc.tile_pool(name="tiny", bufs=1) as tiny_pool,
                    ):
            # ---- constants into SBUF
            cu = const_pool.tile([38, 2 * NB], f32r)
            nc.sync.dma_start(cu[0:6, :], cube3x_d[:, :].bitcast(f32r))
            nc.sync.dma_start(cu[32:38, :], cube3x_d[:, :].bitcast(f32r))
            bias = const_pool.tile([2 * NB, 1], f32)
            nc.sync.dma_start(bias[:, :], bias_d[:, :])
            clampc = const_pool.tile([2 * NB, 2], f32)
            nc.sync.dma_start(clampc[:, :], clampc_d[:, :])
            w2t = const_pool.tile([2 * NB, 2 * NB], f32r)
            nc.sync.dma_start(w2t[:, :], w2_d[:, :].bitcast(f32r))

            # ---- global min/max from the full x (redundant per core)
            # chunked so the reduces overlap the input DMA
            xf = big_pool.tile([128, FULL_COLS], f32)
            NCH = 4
            csz = (FULL_COLS + NCH - 1) // NCH
            pq = tiny_pool.tile([128, 2 * NCH], f32)
            for ci in range(NCH):
                lo = ci * csz
                hi = min(FULL_COLS, lo + csz)
                nc.sync.dma_start(xf[:, lo:hi], x_full[:, lo:hi])
                nc.vector.tensor_reduce(
                    pq[:, ci:ci + 1], xf[:, lo:hi], mybir.AxisListType.X,
                    mybir.AluOpType.min,
                )
                nc.vector.tensor_reduce(
                    pq[:, NCH + ci:NCH + ci + 1], xf[:, lo:hi],
                    mybir.AxisListType.X, mybir.AluOpType.max,
                )
            pm = tiny_pool.tile([128, 2], f32)  # per-partition [-min, max]
            nc.vector.tensor_reduce(
                pm[:, 0:1], pq[:, 0:NCH], mybir.AxisListType.X,
                mybir.AluOpType.min, negate=True,
            )
            nc.vector.tensor_reduce(
                pm[:, 1:2], pq[:, NCH:2 * NCH], mybir.AxisListType.X,
                mybir.AluOpType.max,
            )
            g = tiny_pool.tile([1, 4], f32)  # [min, inv, max, span]
            nc.gpsimd.tensor_reduce(
                g[0:1, 0:1], pm[:, 0:1], mybir.AxisListType.XYZWC,
                mybir.AluOpType.max,
            )
            nc.gpsimd.tensor_reduce(
                g[0:1, 2:3], pm[:, 1:2], mybir.AxisListType.XYZWC,
                mybir.AluOpType.max,
            )
            # g0 currently holds -min: span = (max + (-min)) + 1e-8
            nc.vector.tensor_scalar(
                g[0:1, 3:4], g[0:1, 2:3], g[0:1, 0:1], 1e-8,
                mybir.AluOpType.add, mybir.AluOpType.add,
            )
            nc.vector.reciprocal(g[0:1, 1:2], g[0:1, 3:4])
            # g0 := center = 0.5*span - (-min)  (powers are of y = xn - 0.5)
            nc.vector.scalar_tensor_tensor(
                g[0:1, 0:1], g[0:1, 3:4], 0.5, g[0:1, 0:1],
                mybir.AluOpType.mult, mybir.AluOpType.subtract,
            )
            # broadcast (min, inv) to all partitions via a K=1 matmul
            ones = tiny_pool.tile([1, 128], f32)
            nc.vector.memset(ones[:, :], 1.0)
            muinv_p = gpsum_pool.tile([128, 2], f32, tag="gp")
            nc.tensor.matmul(muinv_p[:, :], ones[:, :], g[0:1, 0:2])
            muinv = tiny_pool.tile([128, 2], f32)
            nc.scalar.copy(muinv[:, :], muinv_p[:, :])

            # ---- power table XP [128, 4*489]: blocks [1 | xn | xn^2 | xn^3]
            xs = big_pool.tile([128, TILE_W], f32)
            nc.sync.dma_start(xs[:, :], x_shard[:, :])
            xp = big_pool.tile([128, 3 * TILE_W], f32)
            W = TILE_W
            nc.vector.tensor_scalar(
                xp[:, 0:W], xs[:, :], muinv[:, 0:1], muinv[:, 1:2],
                mybir.AluOpType.subtract, mybir.AluOpType.mult,
            )
            nc.vector.tensor_mul(xp[:, W:2 * W], xp[:, 0:W], xp[:, 0:W])
            nc.vector.tensor_mul(xp[:, 2 * W:3 * W], xp[:, W:2 * W], xp[:, 0:W])

            # ---- main pipeline: 64 pairs of 490-point tiles (block-diag),
            # two pairs share one 2-bank PSUM tile so clamp/evict batch up
            n_pairs = N_TILES // 2
            BANK = 512
            for g in range(0, n_pairs, GROUP):
                stage = stage_pool.tile([2 * NB, GROUP * W], f32)
                for pb in range(g, min(g + GROUP, n_pairs), 2):
                    sl = (pb - g) * W

                    xr = xrows_pool.tile([38, W], f32r)
                    for h in (0, 1):
                        t0 = 2 * (pb + h)
                        eng = nc.gpsimd if (pb + h) % 2 == 0 else nc.sync
                        eng.dma_start(
                            xr[32 * h:32 * h + 6, :],
                            xp[t0:t0 + 2, :].bitcast(f32r).rearrange(
                                "p (q c) -> p q c", q=3),
                        )

                    gp = gpsum_pool.tile([2 * NB, 2 * BANK], f32)
                    nc.tensor.matmul(gp[:, 0:W], cu[0:6, :], xr[0:6, :])
                    nc.tensor.matmul(
                        gp[:, BANK:BANK + W], cu[32:38, :], xr[32:38, :]
                    )

                    stk = stack_pool.tile([2 * NB, 2 * W], f32r)
                    nc.vector.tensor_scalar(
                        stk[:, :].rearrange("r (p c) -> r p c", c=W),
                        gp[:, :].rearrange("r (p c) -> r p c", c=BANK)[:, :, 0:W],
                        bias[:, :], clampc[:, 0:1],
                        mybir.AluOpType.add, mybir.AluOpType.max,
                    )

                    op = opsum_pool.tile([2 * NB, 2 * BANK], f32)
                    nc.tensor.matmul(op[:, 0:W], w2t[:, :], stk[:, 0:W])
                    nc.tensor.matmul(
                        op[:, BANK:BANK + W], w2t[:, :], stk[:, W:2 * W]
                    )

                    nc.scalar.activation(
                        stage[:, sl:sl + 2 * W].rearrange(
                            "r (p c) -> r p c", c=W),
                        op[:, :].rearrange("r (p c) -> r p c", c=BANK)[:, :, 0:W],
                        mybir.ActivationFunctionType.Copy,
                    )

                # pair-major blocks (even tiles then odd tiles); the host
                # un-interleaves columns, keeping these DMAs fully contiguous
                gw = min(GROUP, n_pairs - g)
                c0 = 2 * g * W
                nc.scalar.dma_start(
                    out_t[:, c0:c0 + gw * W], stage[0:NB, 0:gw * W]
                )
                nc.scalar.dma_start(
                    out_t[:, c0 + gw * W:c0 + 2 * gw * W],
                    stage[NB:2 * NB, 0:gw * W],
                )

    nc.compile()
    return nc


# ------------------------------------------------------------------- driver
def _run(in_maps, trace=False):
    from concourse.bass_utils import run_bass_kernel_spmd

    if "nc" not in _CACHE:
        _CACHE["nc"] = _build_nc()
    return run_bass_kernel_spmd(
        _CACHE["nc"], in_maps, list(range(N_CORES)), trace=trace
    )


def _default_knots():
    inner = np.linspace(0.0, 1.0, NUM_KNOTS - 2 * DEGREE)
    return np.concatenate(
        [np.zeros(DEGREE), inner, np.ones(DEGREE)]
    ).astype(np.float32)


def kernel(x, knots=None, degree=None, _trace=False, _return_results=False, **_):
    x = np.asarray(x, np.float32).reshape(-1)
    assert x.size == N_POINTS
    if knots is None:
        knots = _default_knots()
    cube3x, bias124, clampc, w2, row1 = _make_const_arrays(
        np.asarray(knots, np.float64))

    xf = np.empty(FULL_PAD, np.float32)
    xf[:N_POINTS] = x
    xf[N_POINTS:] = x[0]
    xf = xf.reshape(128, FULL_COLS)

    in_maps = []
    for c in range(N_CORES):
        sh = np.empty(SHARD_PAD, np.float32)
        sh[:SHARD] = x[c * SHARD:(c + 1) * SHARD]
        sh[SHARD:] = x[c * SHARD]
        in_maps.append({
            "x_full": xf,
            "x_shard": sh.reshape(128, TILE_W),
            "cube3x": cube3x,
            "bias124": bias124,
            "clampc": clampc,
            "w2": w2,
        })

    res = _run(in_maps, trace=_trace)
    # device column -> local point index (pair-major group blocks)
    GROUP = 8
    W = TILE_W
    n_pairs = N_TILES // 2
    perm = np.empty(SHARD_PAD, np.int64)
    col = 0
    for g in range(0, n_pairs, GROUP):
        gw = min(GROUP, n_pairs - g)
        for half in (0, 1):
            for pp in range(gw):
                t = 2 * (g + pp) + half
                perm[col:col + W] = t * W + np.arange(W)
                col += W
    out = np.empty((N_POINTS, NB), np.float32)
    full = np.empty((SHARD_PAD, NB), np.float32)
    for c in range(N_CORES):
        full[perm, :] = res.results[c]["out_t"].T
        out[c * SHARD:(c + 1) * SHARD, :] = full[:SHARD]

    # boundary fixup: at xn == 1.0 exactly the reference jumps to the
    # degenerate right-end pieces (s = 64); patch those rows exactly
    mn, mx = x.min(), x.max()
    xn = (x - mn) / ((mx - mn) + np.float32(1e-8))
    at_one = np.nonzero(xn == np.float32(1.0))[0]
    if at_one.size:
        out[at_one, :] = row1.astype(np.float32)[None, :]

    if _return_results:
        return out, res
    return out

